# revision 12
# baseline (speedup 1.0000x reference)
"""AbsoluteLearnedPE kernel v10: host-side transpose/add/bf16-cast,
device is a pure bf16 matmul stream; bf16 output upconverted on host.

Per core (data-parallel over batch): logits = q_b @ E^T + E @ (k_b+E)^T.
Host feeds stripe-major bf16 tensors [KS, 128, DTILES, 512]; device
DMAs are [128,512] chunks in PE-consumption order.

v10: growing-frontier group order. Early DMA delivery is capped around
250GB/s (4-outstanding descriptors per queue, ~16 shared engines), so
the group schedule must keep the byte-requirement curve under it:
  A: (ks0, qt0-3)    needs e/q/kpe s0      = 3MB   by PE-t 13.8us
  B: (ks0, qt4-7)    needs +q s1           = 4MB   by 27.7us
  C: (ks1, qt0-7)    needs +e/kpe s1       = 6MB   by 55.4us
  D: (ks0-1, qt8-15) needs +q s2,s3 +e s2,s3 (lhsT)  by 110.8us
  E: (ks2-3, all qt) needs +kpe s2,s3      = 12MB  by 221us
After the initial 3MB the requirement rate is ~72GB/s — delivery always
stays ahead, so the PE streams back-to-back 216ns matmuls.
Rings: Sync (fast HW) = e/q s0 then all outputs; Scalar (fast HW) =
kpe s0, q s1, e s1, kpe s1, q s2, q s3; GpSimd (slow SW ring) =
e s2, kpe s2, e s3, kpe s3 (nothing needed before t=65us).
Evictions all on Vector; 6 warmup matmuls cover the DMA lead-in.
"""

import numpy as np

B, Q, K, D = 8, 2048, 2048, 1024
DTILES = D // 128     # 8
QT = Q // 128         # 16
KSTRIPE = 512
KS = K // KSTRIPE     # 4
WARM_MMS = 6

_CACHE = {}
TRACE = False


def _build():
    from concourse import bacc
    import concourse.mybir as mybir
    import concourse.tile as tile

    f32 = mybir.dt.float32
    bf16 = mybir.dt.bfloat16

    nc = bacc.Bacc("TRN2", target_bir_lowering=False, debug=False, num_devices=B)
    qTs = nc.dram_tensor("qTs", [KS, 128, DTILES, KSTRIPE], bf16,
                         kind="ExternalInput").ap()
    eTs = nc.dram_tensor("eTs", [KS, 128, DTILES, KSTRIPE], bf16,
                         kind="ExternalInput").ap()
    kpeTs = nc.dram_tensor("kpeTs", [KS, 128, DTILES, KSTRIPE], bf16,
                           kind="ExternalInput").ap()
    out16 = nc.dram_tensor("out16", [Q, K], bf16, kind="ExternalOutput").ap()

    with tile.TileContext(nc) as tc:
        with tc.tile_pool(name="big", bufs=1) as big, \
             tc.tile_pool(name="outp", bufs=8) as outp, \
             tc.tile_pool(name="mps", bufs=8, space="PSUM") as mps:

            q_sb = big.tile([128, KS, DTILES, KSTRIPE], bf16, tag="qT")
            e_sb = big.tile([128, KS, DTILES, KSTRIPE], bf16, tag="eT")
            kpe_sb = big.tile([128, KS, DTILES, KSTRIPE], bf16, tag="kpeT")

            # PE warmup during the DMA lead-in.
            wtile = big.tile([128, KSTRIPE], bf16, tag="warm")
            nc.gpsimd.memset(wtile[:], 0.0)
            wps = mps.tile([128, KSTRIPE], f32, tag="mps")
            for _ in range(WARM_MMS):
                nc.tensor.matmul(wps[:], wtile[:, 0:128], wtile[:],
                                 start=True, stop=True)

            # Round-robin every chunk across all three DMA queues, in
            # frontier-consumption order: keeping all queues loaded is what
            # keeps the shared DMA-engine pool fed (~20GB/s per engine
            # regardless of packet size; one queue alone delivers ~1 chunk
            # per 2.4us).
            rr = {"i": 0}
            engs = [nc.sync, nc.scalar, nc.gpsimd]

            def load(sb, dram, s, d):
                eng = engs[rr["i"] % 3]
                rr["i"] += 1
                eng.dma_start(out=sb[:, s, d, :], in_=dram[s, :, d, :])

            for d in range(DTILES):          # block A: (e,q,kpe) s0 per-d
                load(e_sb, eTs, 0, d)
                load(q_sb, qTs, 0, d)
                load(kpe_sb, kpeTs, 0, d)
            for d in range(DTILES):          # block B: q s1 (lhsT)
                load(q_sb, qTs, 1, d)
            for d in range(DTILES):          # block C: e s1 + kpe s1
                load(e_sb, eTs, 1, d)
                load(kpe_sb, kpeTs, 1, d)
            for s in (2, 3):                 # block D: q + e s2/s3 (lhsT)
                for d in range(DTILES):
                    load(q_sb, qTs, s, d)
                    load(e_sb, eTs, s, d)
            for s in (2, 3):                 # block E: kpe s2/s3
                for d in range(DTILES):
                    load(kpe_sb, kpeTs, s, d)

            def wave(ks, qt_base):
                # 4 groups, d-major interleaved: each delivered d-chunk set
                # unlocks 8 matmuls (4 groups x 2 terms) instead of 2 —
                # keeps PE duty high while the early chunks trickle in.
                qts = [qt_base + j for j in range(4)]
                psos = [mps.tile([128, KSTRIPE], f32, tag="mps",
                                 name=f"pso_{ks}_{qt}") for qt in qts]
                for d in range(DTILES):
                    for j, qt in enumerate(qts):
                        sq, cq = qt // 4, qt % 4
                        qs = slice(cq * 128, (cq + 1) * 128)
                        nc.tensor.matmul(psos[j][:], q_sb[:, sq, d, qs],
                                         e_sb[:, ks, d, :],
                                         start=(d == 0), stop=False)
                    for j, qt in enumerate(qts):
                        sq, cq = qt // 4, qt % 4
                        qs = slice(cq * 128, (cq + 1) * 128)
                        nc.tensor.matmul(psos[j][:], e_sb[:, sq, d, qs],
                                         kpe_sb[:, ks, d, :],
                                         start=False, stop=(d == DTILES - 1))
                for j, qt in enumerate(qts):
                    o_t = outp.tile([128, KSTRIPE], bf16, tag="o_t")
                    nc.vector.tensor_copy(out=o_t[:], in_=psos[j][:])
                    nc.sync.dma_start(
                        out=out16[qt * 128:(qt + 1) * 128,
                                  ks * KSTRIPE:(ks + 1) * KSTRIPE],
                        in_=o_t[:])

            WAVES = [(0, 0), (0, 4), (1, 0), (1, 4),        # A, B, C
                     (0, 8), (1, 8), (0, 12), (1, 12),      # D
                     (2, 0), (2, 4), (2, 8), (2, 12),       # E (kpe s2)
                     (3, 0), (3, 4), (3, 8), (3, 12)]       # E (kpe s3)
            for ks, qt_base in WAVES:
                wave(ks, qt_base)
    nc.compile()
    return nc


def _stripe_major(x16: np.ndarray) -> np.ndarray:
    # [D, K] -> [KS, 128, DTILES, 512] with [s, p, d, c] = x[d*128+p, s*512+c]
    return np.ascontiguousarray(
        x16.reshape(DTILES, 128, KS, KSTRIPE).transpose(2, 1, 0, 3))


def kernel(q: np.ndarray, k: np.ndarray, embed: np.ndarray) -> np.ndarray:
    import ml_dtypes
    from concourse.bass_utils import run_bass_kernel_spmd

    if "nc" not in _CACHE:
        _CACHE["nc"] = _build()
    nc = _CACHE["nc"]

    bf = ml_dtypes.bfloat16
    e = np.asarray(embed[:K], dtype=np.float32)
    eTs = _stripe_major(e.T.astype(bf))
    in_maps = []
    for b in range(B):
        qTs = _stripe_major(np.asarray(q[b], dtype=np.float32).T.astype(bf))
        kpeTs = _stripe_major((np.asarray(k[b], dtype=np.float32) + e).T.astype(bf))
        in_maps.append({"qTs": qTs, "eTs": eTs, "kpeTs": kpeTs})
    res = run_bass_kernel_spmd(nc, in_maps, core_ids=list(range(B)), trace=TRACE)
    _CACHE["last_result"] = res
    return np.stack([res.results[b]["out16"].astype(np.float32) for b in range(B)])


# revision 13
# speedup vs baseline: 1.0014x; 1.0014x over previous
"""AbsoluteLearnedPE kernel v10: host-side transpose/add/bf16-cast,
device is a pure bf16 matmul stream; bf16 output upconverted on host.

Per core (data-parallel over batch): logits = q_b @ E^T + E @ (k_b+E)^T.
Host feeds stripe-major bf16 tensors [KS, 128, DTILES, 512]; device
DMAs are [128,512] chunks in PE-consumption order.

v10: growing-frontier group order. Early DMA delivery is capped around
250GB/s (4-outstanding descriptors per queue, ~16 shared engines), so
the group schedule must keep the byte-requirement curve under it:
  A: (ks0, qt0-3)    needs e/q/kpe s0      = 3MB   by PE-t 13.8us
  B: (ks0, qt4-7)    needs +q s1           = 4MB   by 27.7us
  C: (ks1, qt0-7)    needs +e/kpe s1       = 6MB   by 55.4us
  D: (ks0-1, qt8-15) needs +q s2,s3 +e s2,s3 (lhsT)  by 110.8us
  E: (ks2-3, all qt) needs +kpe s2,s3      = 12MB  by 221us
After the initial 3MB the requirement rate is ~72GB/s — delivery always
stays ahead, so the PE streams back-to-back 216ns matmuls.
Rings: Sync (fast HW) = e/q s0 then all outputs; Scalar (fast HW) =
kpe s0, q s1, e s1, kpe s1, q s2, q s3; GpSimd (slow SW ring) =
e s2, kpe s2, e s3, kpe s3 (nothing needed before t=65us).
Evictions all on Vector; 6 warmup matmuls cover the DMA lead-in.
"""

import numpy as np

B, Q, K, D = 8, 2048, 2048, 1024
DTILES = D // 128     # 8
QT = Q // 128         # 16
KSTRIPE = 512
KS = K // KSTRIPE     # 4
WARM_MMS = 12

_CACHE = {}
TRACE = False


def _build():
    from concourse import bacc
    import concourse.mybir as mybir
    import concourse.tile as tile

    f32 = mybir.dt.float32
    bf16 = mybir.dt.bfloat16

    nc = bacc.Bacc("TRN2", target_bir_lowering=False, debug=False, num_devices=B)
    qTs = nc.dram_tensor("qTs", [KS, 128, DTILES, KSTRIPE], bf16,
                         kind="ExternalInput").ap()
    eTs = nc.dram_tensor("eTs", [KS, 128, DTILES, KSTRIPE], bf16,
                         kind="ExternalInput").ap()
    kpeTs = nc.dram_tensor("kpeTs", [KS, 128, DTILES, KSTRIPE], bf16,
                           kind="ExternalInput").ap()
    out16 = nc.dram_tensor("out16", [Q, K], bf16, kind="ExternalOutput").ap()

    with tile.TileContext(nc) as tc:
        with tc.tile_pool(name="big", bufs=1) as big, \
             tc.tile_pool(name="outp", bufs=8) as outp, \
             tc.tile_pool(name="mps", bufs=8, space="PSUM") as mps:

            q_sb = big.tile([128, KS, DTILES, KSTRIPE], bf16, tag="qT")
            e_sb = big.tile([128, KS, DTILES, KSTRIPE], bf16, tag="eT")
            kpe_sb = big.tile([128, KS, DTILES, KSTRIPE], bf16, tag="kpeT")

            # PE warmup during the DMA lead-in.
            wtile = big.tile([128, KSTRIPE], bf16, tag="warm")
            nc.gpsimd.memset(wtile[:], 0.0)
            wps = mps.tile([128, KSTRIPE], f32, tag="mps")
            for _ in range(WARM_MMS):
                nc.tensor.matmul(wps[:], wtile[:, 0:128], wtile[:],
                                 start=True, stop=True)

            # Round-robin every chunk across all three DMA queues, in
            # frontier-consumption order: keeping all queues loaded is what
            # keeps the shared DMA-engine pool fed (~20GB/s per engine
            # regardless of packet size; one queue alone delivers ~1 chunk
            # per 2.4us).
            rr = {"i": 0}
            engs = [nc.sync, nc.scalar, nc.gpsimd]

            def load(sb, dram, s, d):
                eng = engs[rr["i"] % 3]
                rr["i"] += 1
                eng.dma_start(out=sb[:, s, d, :], in_=dram[s, :, d, :])

            for d in range(DTILES):          # block A: (e,q,kpe) s0 per-d
                load(e_sb, eTs, 0, d)
                load(q_sb, qTs, 0, d)
                load(kpe_sb, kpeTs, 0, d)
            for d in range(DTILES):          # block B: q s1 (lhsT)
                load(q_sb, qTs, 1, d)
            for d in range(DTILES):          # block C: e s1 + kpe s1
                load(e_sb, eTs, 1, d)
                load(kpe_sb, kpeTs, 1, d)
            for s in (2, 3):                 # block D: q + e s2/s3 (lhsT)
                for d in range(DTILES):
                    load(q_sb, qTs, s, d)
                    load(e_sb, eTs, s, d)
            for s in (2, 3):                 # block E: kpe s2/s3
                for d in range(DTILES):
                    load(kpe_sb, kpeTs, s, d)

            def wave(ks, qt_base):
                # 4 groups, d-major interleaved: each delivered d-chunk set
                # unlocks 8 matmuls (4 groups x 2 terms) instead of 2 —
                # keeps PE duty high while the early chunks trickle in.
                qts = [qt_base + j for j in range(4)]
                psos = [mps.tile([128, KSTRIPE], f32, tag="mps",
                                 name=f"pso_{ks}_{qt}") for qt in qts]
                for d in range(DTILES):
                    for j, qt in enumerate(qts):
                        sq, cq = qt // 4, qt % 4
                        qs = slice(cq * 128, (cq + 1) * 128)
                        nc.tensor.matmul(psos[j][:], q_sb[:, sq, d, qs],
                                         e_sb[:, ks, d, :],
                                         start=(d == 0), stop=False)
                    for j, qt in enumerate(qts):
                        sq, cq = qt // 4, qt % 4
                        qs = slice(cq * 128, (cq + 1) * 128)
                        nc.tensor.matmul(psos[j][:], e_sb[:, sq, d, qs],
                                         kpe_sb[:, ks, d, :],
                                         start=False, stop=(d == DTILES - 1))
                for j, qt in enumerate(qts):
                    o_t = outp.tile([128, KSTRIPE], bf16, tag="o_t")
                    nc.vector.tensor_copy(out=o_t[:], in_=psos[j][:])
                    nc.sync.dma_start(
                        out=out16[qt * 128:(qt + 1) * 128,
                                  ks * KSTRIPE:(ks + 1) * KSTRIPE],
                        in_=o_t[:])

            WAVES = [(0, 0), (0, 4), (1, 0), (1, 4),        # A, B, C
                     (0, 8), (1, 8), (0, 12), (1, 12),      # D
                     (2, 0), (2, 4), (2, 8), (2, 12),       # E (kpe s2)
                     (3, 0), (3, 4), (3, 8), (3, 12)]       # E (kpe s3)
            for ks, qt_base in WAVES:
                wave(ks, qt_base)
    nc.compile()
    return nc


def _stripe_major(x16: np.ndarray) -> np.ndarray:
    # [D, K] -> [KS, 128, DTILES, 512] with [s, p, d, c] = x[d*128+p, s*512+c]
    return np.ascontiguousarray(
        x16.reshape(DTILES, 128, KS, KSTRIPE).transpose(2, 1, 0, 3))


def kernel(q: np.ndarray, k: np.ndarray, embed: np.ndarray) -> np.ndarray:
    import ml_dtypes
    from concourse.bass_utils import run_bass_kernel_spmd

    if "nc" not in _CACHE:
        _CACHE["nc"] = _build()
    nc = _CACHE["nc"]

    bf = ml_dtypes.bfloat16
    e = np.asarray(embed[:K], dtype=np.float32)
    eTs = _stripe_major(e.T.astype(bf))
    in_maps = []
    for b in range(B):
        qTs = _stripe_major(np.asarray(q[b], dtype=np.float32).T.astype(bf))
        kpeTs = _stripe_major((np.asarray(k[b], dtype=np.float32) + e).T.astype(bf))
        in_maps.append({"qTs": qTs, "eTs": eTs, "kpeTs": kpeTs})
    res = run_bass_kernel_spmd(nc, in_maps, core_ids=list(range(B)), trace=TRACE)
    _CACHE["last_result"] = res
    return np.stack([res.results[b]["out16"].astype(np.float32) for b in range(B)])
